# revision 12
# baseline (speedup 1.0000x reference)
"""Multi-head attention on 8 Trainium2 NeuronCores (Bass/Tile).

Sharding: core c handles batch b = c//2 and head-half hh = c%2
(heads 8*hh..8*hh+7, i.e. 512 of the 1024 hidden columns).
Each core computes its half-head attention plus the partial output
projection through Wo[:, cols_half]; the host sums the two partials
per batch and adds bo.

Per-core device program: see build_nc() docstring.
"""

import numpy as np
from contextlib import ExitStack

import ml_dtypes

import concourse.bacc as bacc
import concourse.mybir as mybir
import concourse.tile as tile
from concourse.bass_utils import run_bass_kernel_spmd

F32 = mybir.dt.float32
AF = mybir.ActivationFunctionType
MUL = mybir.AluOpType.mult

B, S, D, H = 4, 2048, 1024, 16
HD = 64
P = 128
JC = 512          # head-cols per core
DC = D // P       # 8 d-chunks
JCH = JC // P     # 4 j-chunks
SC = S // P       # 16 s/k chunks of 128
NQ = 512          # q processed in chunks of 512
NQC = S // NQ     # 4
KG = 2            # k-chunks per group (DVE/ACT op width 1024)
NKG = SC // KG    # 8
JCA = 8 * (HD + 1)  # 520: v-proj output cols, ones column per head baked in

MASKDT = mybir.dt.bfloat16
_NP_MASKDT = ml_dtypes.bfloat16


def build_nc(DT=mybir.dt.float32r, maskdt=MASKDT):
    """One NeuronCore's program. All matmul operands bitcast to DT."""
    nc = bacc.Bacc("TRN2", target_bir_lowering=False, debug=False)

    def mm(out, lhsT, rhs, **kw):
        nc.tensor.matmul(out, lhsT, rhs, **kw)

    xqT = nc.dram_tensor("xqT", [D, S], DT, kind="ExternalInput").ap()
    xkT = nc.dram_tensor("xkT", [D, S], DT, kind="ExternalInput").ap()
    xvT = nc.dram_tensor("xvT", [D, S], DT, kind="ExternalInput").ap()
    wqT = nc.dram_tensor("wqT", [D, JC], DT, kind="ExternalInput").ap()
    wkT = nc.dram_tensor("wkT", [D, JC], DT, kind="ExternalInput").ap()
    wvT = nc.dram_tensor("wvT", [D, JCA], DT, kind="ExternalInput").ap()
    bq = nc.dram_tensor("bq", [JC], F32, kind="ExternalInput").ap()
    bk = nc.dram_tensor("bk", [JC], F32, kind="ExternalInput").ap()
    bv = nc.dram_tensor("bv", [JCA], F32, kind="ExternalInput").ap()
    woT = nc.dram_tensor("woT", [JC, D], DT, kind="ExternalInput").ap()
    maskT = nc.dram_tensor("maskT", [S, S], maskdt, kind="ExternalInput").ap()
    outp = nc.dram_tensor("outp", [S, D], F32, kind="ExternalOutput").ap()

    with (
        tile.TileContext(nc) as tc,
        nc.allow_low_precision(reason="float32r operand storage for PE fast path"),
        ExitStack() as persist,
    ):
        # ---- long-lived SBUF ----
        pers = persist.enter_context(tc.tile_pool(name="pers", bufs=1))
        qT_sb = pers.tile([P, JCH, S], DT)          # [j%128, jc, s]
        kT_sb = pers.tile([P, JCH, S], DT)
        v_sb = pers.tile([P, SC, 8, HD + 1], DT)    # [s%128, sc, head, hd+ones]
        outnT_sb = pers.tile([P, JCH, S], DT)       # normalized head_out.T
        woT_sb = pers.tile([P, JCH, D], DT)
        ones_c = pers.tile([1, P], F32)              # K=1 lhsT for bias/broadcast
        bv_row = pers.tile([1, JCA], F32)
        bq_sb = pers.tile([P, JCH], F32)
        bk_sb = pers.tile([P, JCH], F32)

        nc.vector.memset(ones_c[:], 1.0)
        nc.sync.dma_start(woT_sb[:], woT.rearrange("(c p) j -> p c j", p=P))
        nc.sync.dma_start(bv_row[:], bv.rearrange("(o j) -> o j", o=1))
        nc.sync.dma_start(bq_sb[:], bq.rearrange("(c p) -> p c", p=P))
        nc.sync.dma_start(bk_sb[:], bk.rearrange("(c p) -> p c", p=P))

        # ---- stage A: projections ----
        with ExitStack() as proj:
            xpool = proj.enter_context(tc.tile_pool(name="xp", bufs=2))
            wpool = proj.enter_context(tc.tile_pool(name="wp", bufs=1))
            pp = proj.enter_context(tc.tile_pool(name="pp", bufs=4, space="PSUM"))
            ppv = proj.enter_context(tc.tile_pool(name="ppv", bufs=2, space="PSUM"))

            # q and k: out chunks [j 128, s 512] = wT.T @ xT  (+bias on ACT copy)
            for name, xT, wT, b_sb, dst in (
                ("q", xqT, wqT, bq_sb, qT_sb),
                ("k", xkT, wkT, bk_sb, kT_sb),
            ):
                w_sb = wpool.tile([P, DC, JC], DT, tag="w")
                nc.sync.dma_start(w_sb[:], wT.rearrange("(dc p) j -> p dc j", p=P))
                for s4 in range(NQC):
                    xblk = xpool.tile([P, DC, NQ], DT, tag="x")
                    nc.sync.dma_start(
                        xblk[:],
                        xT[:, s4 * NQ : (s4 + 1) * NQ].rearrange(
                            "(dc p) s -> p dc s", p=P
                        ),
                    )
                    for j in range(JCH):
                        ps = pp.tile([P, NQ], F32, tag="pp")
                        for d in range(DC):
                            mm(
                                ps[:],
                                w_sb[:, d, j * P : (j + 1) * P],
                                xblk[:, d, :],
                                start=(d == 0),
                                stop=(d == DC - 1),
                            )
                        nc.scalar.activation(
                            dst[:, j, s4 * NQ : (s4 + 1) * NQ],
                            ps[:],
                            AF.Identity,
                            bias=b_sb[:, j : j + 1],
                        )

            # v: out chunks [s 128, j 512] = xvT.T @ wvT (+bv via K=1 matmul)
            w_sb = wpool.tile([P, DC, JCA], DT, tag="w")
            nc.sync.dma_start(w_sb[:], wvT.rearrange("(dc p) j -> p dc j", p=P))
            for s4 in range(NQC):
                xblk = xpool.tile([P, DC, NQ], DT, tag="x")
                nc.sync.dma_start(
                    xblk[:],
                    xvT[:, s4 * NQ : (s4 + 1) * NQ].rearrange("(dc p) s -> p dc s", p=P),
                )
                for s16 in range(4):
                    sc = s4 * 4 + s16
                    ps = ppv.tile([P, 2, NQ], F32, tag="ppv")
                    HJ = JCA // 2  # 260
                    for half in range(2):
                        for d in range(DC):
                            mm(
                                ps[:, half, 0:HJ],
                                xblk[:, d, s16 * P : (s16 + 1) * P],
                                w_sb[:, d, half * HJ : (half + 1) * HJ],
                                start=(d == 0),
                                stop=False,
                            )
                        mm(
                            ps[:, half, 0:HJ],
                            ones_c[:],
                            bv_row[:, half * HJ : (half + 1) * HJ],
                            start=False,
                            stop=True,
                        )
                    nc.scalar.activation(
                        v_sb[:, sc].rearrange("p (a b) e -> p a (b e)", a=2),
                        ps[:, :, 0:HJ],
                        AF.Copy,
                    )

        # ---- stage B: attention ----
        with ExitStack() as attn:
            mpool = attn.enter_context(tc.tile_pool(name="mp", bufs=1))
            epool = attn.enter_context(tc.tile_pool(name="ep", bufs=3))
            spool = attn.enter_context(tc.tile_pool(name="sp", bufs=3, space="PSUM"))
            opool = attn.enter_context(tc.tile_pool(name="op", bufs=2, space="PSUM"))
            rpool = attn.enter_context(tc.tile_pool(name="rp", bufs=2))

            for qc in range(NQC):
                mblk = mpool.tile([P, SC, NQ], MASKDT, tag="m")
                nc.sync.dma_start(
                    mblk[:],
                    maskT[:, qc * NQ : (qc + 1) * NQ].rearrange(
                        "(kc p) q -> p kc q", p=P
                    ),
                )
                for hp in range(4):
                    o_ps0 = opool.tile([HD + 1, NQ], F32, tag="ot")
                    o_ps1 = opool.tile([HD + 1, NQ], F32, tag="ot")
                    o_ps = (o_ps0, o_ps1)
                    for kg in range(NKG):
                        e_ts = []
                        for hi in range(2):
                            h = 2 * hp + hi
                            r0 = (h % 2) * HD
                            ch = h // 2
                            sc_ps = spool.tile([P, KG, NQ], F32, tag="sc")
                            for k2 in range(KG):
                                kc = kg * KG + k2
                                mm(
                                    sc_ps[:, k2, :],
                                    kT_sb[r0 : r0 + HD, ch, kc * P : (kc + 1) * P],
                                    qT_sb[r0 : r0 + HD, ch, qc * NQ : (qc + 1) * NQ],
                                    start=True,
                                    stop=True,
                                )
                            me = epool.tile([P, KG, NQ], F32, tag="me")
                            nc.vector.tensor_tensor(
                                me[:],
                                sc_ps[:],
                                mblk[:, kg * KG : (kg + 1) * KG, :],
                                MUL,
                            )
                            et = epool.tile([P, KG, NQ], DT, tag="et")
                            nc.scalar.activation(et[:], me[:], AF.Exp)
                            e_ts.append(et)
                        for hi in range(2):
                            h = 2 * hp + hi
                            for k2 in range(KG):
                                kc = kg * KG + k2
                                mm(
                                    o_ps[hi][:],
                                    v_sb[:, kc, h, :],
                                    e_ts[hi][:, k2, :],
                                    start=(kg == 0 and k2 == 0),
                                    stop=(kg == NKG - 1 and k2 == KG - 1),
                                )
                    for hi in range(2):
                        h = 2 * hp + hi
                        r0 = (h % 2) * HD
                        ch = h // 2
                        recip = rpool.tile([1, NQ], F32, tag="rc")
                        nc.vector.reciprocal(recip[:], o_ps[hi][HD : HD + 1, :])
                        rb_ps = spool.tile([HD, NQ], F32, tag="sc")
                        mm(rb_ps[:], ones_c[:, 0:HD], recip[:], start=True, stop=True)
                        rb_sb = rpool.tile([HD, NQ], F32, tag="rb")
                        nc.scalar.activation(rb_sb[:], rb_ps[:], AF.Copy)
                        nc.vector.tensor_tensor(
                            outnT_sb[r0 : r0 + HD, ch, qc * NQ : (qc + 1) * NQ],
                            o_ps[hi][0:HD, :],
                            rb_sb[:],
                            MUL,
                        )

        # ---- stage C: output projection ----
        with ExitStack() as wo:
            fpool = wo.enter_context(tc.tile_pool(name="fp", bufs=2, space="PSUM"))
            obuf = wo.enter_context(tc.tile_pool(name="ob", bufs=2))
            for sc in range(SC):
                fps = fpool.tile([P, 2, NQ], F32, tag="f")
                for n2 in range(2):
                    for cc in range(JCH):
                        mm(
                            fps[:, n2, :],
                            outnT_sb[:, cc, sc * P : (sc + 1) * P],
                            woT_sb[:, cc, n2 * NQ : (n2 + 1) * NQ],
                            start=(cc == 0),
                            stop=(cc == JCH - 1),
                        )
                ob = obuf.tile([P, D], F32, tag="o")
                nc.scalar.activation(ob[:], fps[:].rearrange("p a b -> p (a b)"), AF.Copy)
                nc.sync.dma_start(outp[sc * P : (sc + 1) * P, :], ob[:])

    nc.compile()
    return nc


def _augment_wv(WvJ, f):
    # [512, 1024] row-slice -> transposed + per-head ones column -> [1024, 520]
    out = np.zeros((D, JCA), dtype=f)
    wt = WvJ.T  # [1024, 512]
    for h in range(8):
        out[:, h * (HD + 1) : h * (HD + 1) + HD] = wt[:, h * HD : (h + 1) * HD]
    return out


def _augment_bv(bvJ, f):
    out = np.zeros(JCA, dtype=f)
    for h in range(8):
        out[h * (HD + 1) : h * (HD + 1) + HD] = bvJ[h * HD : (h + 1) * HD]
        out[h * (HD + 1) + HD] = 1.0
    return out


def _prep_in_maps(query, key, value, mask, Wq, bq, Wk, bk, Wv, bv, Wo, bo):
    f = np.float32
    per_batch = []
    for b in range(B):
        per_batch.append(
            dict(
                xqT=np.ascontiguousarray(query[b].T, dtype=f),
                xkT=np.ascontiguousarray(key[b].T, dtype=f),
                xvT=np.ascontiguousarray(value[b].T, dtype=f),
                maskT=np.ascontiguousarray(mask[b, 0].T).astype(_NP_MASKDT),
            )
        )
    per_half = []
    for hh in range(2):
        J = slice(JC * hh, JC * (hh + 1))
        per_half.append(
            dict(
                wqT=np.ascontiguousarray(Wq[J].T, dtype=f),
                wkT=np.ascontiguousarray(Wk[J].T, dtype=f),
                wvT=_augment_wv(Wv[J], f),
                bq=np.ascontiguousarray(bq[J], dtype=f),
                bk=np.ascontiguousarray(bk[J], dtype=f),
                bv=_augment_bv(bv[J], f),
                woT=np.ascontiguousarray(Wo[:, J].T, dtype=f),
            )
        )
    in_maps = []
    for c in range(8):
        m = dict(per_batch[c // 2])
        m.update(per_half[c % 2])
        in_maps.append(m)
    return in_maps


_NC_CACHE = {}


def _get_nc(dt_name="float32r"):
    if dt_name not in _NC_CACHE:
        _NC_CACHE[dt_name] = build_nc(DT=getattr(mybir.dt, dt_name))
    return _NC_CACHE[dt_name]


# ---------------------------------------------------------------------------
# Cached PJRT runner.  Mirrors run_bass_kernel_spmd's axon redirect
# (bass2jax.run_bass_via_pjrt) but builds the jitted shard_map once per
# (dt_name, n_execs) so repeat kernel() calls skip re-tracing, and supports
# chaining n_execs sequential NEFF executions inside one program so test.py
# can measure per-execution hardware time as a slope (dispatch overhead
# cancels).
# ---------------------------------------------------------------------------
_RUNNER_CACHE = {}


def _get_runner(dt_name="float32r", n_execs=1):
    key = (dt_name, n_execs)
    if key in _RUNNER_CACHE:
        return _RUNNER_CACHE[key]

    import jax
    from jax.sharding import Mesh, PartitionSpec
    from jax.experimental.shard_map import shard_map
    from concourse import bass2jax
    from concourse.bass2jax import _bass_exec_p

    bass2jax.install_neuronx_cc_hook()
    nc = _get_nc(dt_name)
    partition_name = nc.partition_id_tensor.name if nc.partition_id_tensor else None

    in_names = []
    out_names = []
    out_avals = []
    for alloc in nc.m.functions[0].allocations:
        if not isinstance(alloc, mybir.MemoryLocationSet):
            continue
        name = alloc.memorylocations[0].name
        if alloc.kind == "ExternalInput":
            if name != partition_name:
                in_names.append(name)
        elif alloc.kind == "ExternalOutput":
            out_names.append(name)
            out_avals.append(
                jax.core.ShapedArray(tuple(alloc.tensor_shape), mybir.dt.np(alloc.dtype))
            )
    n_params = len(in_names)
    n_outs = len(out_avals)
    all_in_names = tuple(in_names + out_names)
    if partition_name is not None:
        all_in_names = all_in_names + (partition_name,)

    def _body(*args):
        params = list(args[:n_params])
        zeros = list(args[n_params:])
        pid = (
            [bass2jax.partition_id_tensor()] if partition_name is not None else []
        )
        outs = None
        for _ in range(n_execs):
            outs = _bass_exec_p.bind(
                *params,
                *zeros,
                *pid,
                out_avals=tuple(out_avals),
                in_names=all_in_names,
                out_names=tuple(out_names),
                lowering_input_output_aliases=(),
                sim_require_finite=True,
                sim_require_nnan=True,
                nc=nc,
            )
            zeros = list(outs)
        return tuple(outs)

    devices = jax.devices()[:8]
    mesh = Mesh(np.asarray(devices), ("core",))
    in_specs = (PartitionSpec("core"),) * (n_params + n_outs)
    out_specs = (PartitionSpec("core"),) * n_outs
    fn = jax.jit(
        shard_map(_body, mesh=mesh, in_specs=in_specs, out_specs=out_specs,
                  check_rep=False),
        keep_unused=True,
    )
    runner = (fn, in_names, out_names, out_avals)
    _RUNNER_CACHE[key] = runner
    return runner


def _concat_inputs(in_maps, in_names, out_avals, out_names):
    args = []
    for name in in_names:
        args.append(np.concatenate([np.asarray(m[name]) for m in in_maps], axis=0))
    for i, name in enumerate(out_names):
        z = out_avals[i]
        args.append(np.zeros((8 * z.shape[0], *z.shape[1:]), z.dtype))
    return args


def run(inputs, dt_name="float32r"):
    """Returns (full_output [B,S,D] f32, per-core outp list)."""
    fn, in_names, out_names, out_avals = _get_runner(dt_name, 1)
    in_maps = _prep_in_maps(**inputs)
    args = _concat_inputs(in_maps, in_names, out_avals, out_names)
    out_arrs = fn(*args)
    i = out_names.index("outp")
    per_core = np.asarray(out_arrs[i]).reshape(8, S, D)
    bo = np.asarray(inputs["bo"], dtype=np.float32)
    out = np.empty((B, S, D), dtype=np.float32)
    for b in range(B):
        out[b] = per_core[2 * b] + per_core[2 * b + 1] + bo
    return out, per_core


def bench(inputs, dt_name="float32r", n_execs=4, iters=6):
    """Time n_execs chained NEFF executions; returns list of wall times (s)."""
    import time as _time
    import jax
    fn, in_names, out_names, out_avals = _get_runner(dt_name, n_execs)
    in_maps = _prep_in_maps(**inputs)
    args = _concat_inputs(in_maps, in_names, out_avals, out_names)
    dargs = [jax.device_put(a) for a in args]
    times = []
    for _ in range(iters):
        t0 = _time.perf_counter()
        outs = fn(*dargs)
        jax.block_until_ready(outs)
        times.append(_time.perf_counter() - t0)
    return times


_TRIVIAL = {}


def _get_trivial():
    """A minimal 8-core NEFF (one small copy) to measure dispatch overhead."""
    if "nc" in _TRIVIAL:
        return _TRIVIAL["nc"]
    nc = bacc.Bacc("TRN2", target_bir_lowering=False, debug=False)
    a = nc.dram_tensor("a", [P, NQ], F32, kind="ExternalInput").ap()
    o = nc.dram_tensor("o", [P, NQ], F32, kind="ExternalOutput").ap()
    with tile.TileContext(nc) as tc, ExitStack() as ctx:
        sb = ctx.enter_context(tc.tile_pool(name="sb", bufs=1))
        t = sb.tile([P, NQ], F32)
        nc.sync.dma_start(t[:], a[:])
        nc.sync.dma_start(o[:], t[:])
    nc.compile()
    _TRIVIAL["nc"] = nc
    return nc


def bench_trivial(iters=8):
    import time as _time
    import jax
    from jax.sharding import Mesh, PartitionSpec
    from jax.experimental.shard_map import shard_map
    from concourse import bass2jax
    from concourse.bass2jax import _bass_exec_p

    if "fn" not in _TRIVIAL:
        bass2jax.install_neuronx_cc_hook()
        nc = _get_trivial()
        partition_name = nc.partition_id_tensor.name if nc.partition_id_tensor else None
        out_avals = (jax.core.ShapedArray((P, NQ), np.float32),)
        all_in = ("a", "o") + ((partition_name,) if partition_name else ())

        def _body(a, z):
            pid = [bass2jax.partition_id_tensor()] if partition_name else []
            return tuple(_bass_exec_p.bind(
                a, z, *pid,
                out_avals=out_avals,
                in_names=all_in,
                out_names=("o",),
                lowering_input_output_aliases=(),
                sim_require_finite=True,
                sim_require_nnan=True,
                nc=nc,
            ))

        devices = jax.devices()[:8]
        mesh = Mesh(np.asarray(devices), ("core",))
        fn = jax.jit(
            shard_map(
                _body, mesh=mesh,
                in_specs=(PartitionSpec("core"),) * 2,
                out_specs=(PartitionSpec("core"),),
                check_rep=False,
            ),
            keep_unused=True,
        )
        _TRIVIAL["fn"] = fn
    fn = _TRIVIAL["fn"]
    import jax
    A = jax.device_put(np.zeros((8 * P, NQ), np.float32))
    Z = jax.device_put(np.zeros((8 * P, NQ), np.float32))
    times = []
    for _ in range(iters):
        t0 = _time.perf_counter()
        outs = fn(A, Z)
        jax.block_until_ready(outs)
        times.append(_time.perf_counter() - t0)
    return times


def kernel(**inputs):
    out, _ = run(inputs)
    return out


# revision 15
# speedup vs baseline: 10.2761x; 10.2761x over previous
"""Multi-head attention on 8 Trainium2 NeuronCores (Bass/Tile).

Sharding: core c handles batch b = c//2 and head-half hh = c%2
(heads 8*hh..8*hh+7, i.e. 512 of the 1024 hidden columns).
Each core computes its half-head attention plus the partial output
projection through Wo[:, cols_half]; the host sums the two partials
per batch and adds bo.

Per-core device program: see build_nc() docstring.
"""

import numpy as np
from contextlib import ExitStack

import ml_dtypes

import concourse.bacc as bacc
import concourse.mybir as mybir
import concourse.tile as tile
from concourse.bass_utils import run_bass_kernel_spmd

F32 = mybir.dt.float32
AF = mybir.ActivationFunctionType
MUL = mybir.AluOpType.mult

B, S, D, H = 4, 2048, 1024, 16
HD = 64
P = 128
JC = 512          # head-cols per core
DC = D // P       # 8 d-chunks
JCH = JC // P     # 4 j-chunks
SC = S // P       # 16 s/k chunks of 128
NQ = 512          # q processed in chunks of 512
NQC = S // NQ     # 4
KG = 2            # k-chunks per group (DVE/ACT op width 1024)
NKG = SC // KG    # 8
JCA = 8 * (HD + 1)  # 520: v-proj output cols, ones column per head baked in

MASKDT = mybir.dt.bfloat16
_NP_MASKDT = ml_dtypes.bfloat16


def build_nc(DT=mybir.dt.float32r, maskdt=MASKDT, n_reps=1):
    """One NeuronCore's program. All matmul operands bitcast to DT."""
    nc = bacc.Bacc("TRN2", target_bir_lowering=False, debug=False)

    def mm(out, lhsT, rhs, **kw):
        nc.tensor.matmul(out, lhsT, rhs, **kw)

    xqT = nc.dram_tensor("xqT", [D, S], DT, kind="ExternalInput").ap()
    xkT = nc.dram_tensor("xkT", [D, S], DT, kind="ExternalInput").ap()
    xvT = nc.dram_tensor("xvT", [D, S], DT, kind="ExternalInput").ap()
    wqT = nc.dram_tensor("wqT", [D, JC], DT, kind="ExternalInput").ap()
    wkT = nc.dram_tensor("wkT", [D, JC], DT, kind="ExternalInput").ap()
    wvT = nc.dram_tensor("wvT", [D, JCA], DT, kind="ExternalInput").ap()
    bq = nc.dram_tensor("bq", [JC], F32, kind="ExternalInput").ap()
    bk = nc.dram_tensor("bk", [JC], F32, kind="ExternalInput").ap()
    bv = nc.dram_tensor("bv", [JCA], F32, kind="ExternalInput").ap()
    woT = nc.dram_tensor("woT", [JC, D], DT, kind="ExternalInput").ap()
    maskT = nc.dram_tensor("maskT", [S, S], maskdt, kind="ExternalInput").ap()
    outp = nc.dram_tensor("outp", [S, D], F32, kind="ExternalOutput").ap()

    with (
        tile.TileContext(nc) as tc,
        nc.allow_low_precision(reason="float32r operand storage for PE fast path"),
        ExitStack() as persist,
    ):
        # ---- long-lived SBUF ----
        pers = persist.enter_context(tc.tile_pool(name="pers", bufs=1))
        qT_sb = pers.tile([P, JCH, S], DT)          # [j%128, jc, s]
        kT_sb = pers.tile([P, JCH, S], DT)
        v_sb = pers.tile([P, SC, 8, HD + 1], DT)    # [s%128, sc, head, hd+ones]
        outnT_sb = pers.tile([P, JCH, S], DT)       # normalized head_out.T
        woT_sb = pers.tile([P, JCH, D], DT)
        ones_c = pers.tile([1, P], F32)              # K=1 lhsT for bias/broadcast
        bv_row = pers.tile([1, JCA], F32)
        bq_sb = pers.tile([P, JCH], F32)
        bk_sb = pers.tile([P, JCH], F32)

        nc.vector.memset(ones_c[:], 1.0)
        nc.sync.dma_start(woT_sb[:], woT.rearrange("(c p) j -> p c j", p=P))
        nc.sync.dma_start(bv_row[:], bv.rearrange("(o j) -> o j", o=1))
        nc.sync.dma_start(bq_sb[:], bq.rearrange("(c p) -> p c", p=P))
        nc.sync.dma_start(bk_sb[:], bk.rearrange("(c p) -> p c", p=P))

        # ---- stage A: projections ----
        with ExitStack() as proj:
            xpool = proj.enter_context(tc.tile_pool(name="xp", bufs=2))
            wpool = proj.enter_context(tc.tile_pool(name="wp", bufs=1))
            pp = proj.enter_context(tc.tile_pool(name="pp", bufs=4, space="PSUM"))
            ppv = proj.enter_context(tc.tile_pool(name="ppv", bufs=2, space="PSUM"))
            if n_reps > 1:
                rep_a = proj.enter_context(tc.For_i(0, n_reps, 1))

            # q and k: out chunks [j 128, s 512] = wT.T @ xT  (+bias on ACT copy)
            for name, xT, wT, b_sb, dst in (
                ("q", xqT, wqT, bq_sb, qT_sb),
                ("k", xkT, wkT, bk_sb, kT_sb),
            ):
                w_sb = wpool.tile([P, DC, JC], DT, tag="w")
                nc.sync.dma_start(w_sb[:], wT.rearrange("(dc p) j -> p dc j", p=P))
                for s4 in range(NQC):
                    xblk = xpool.tile([P, DC, NQ], DT, tag="x")
                    nc.sync.dma_start(
                        xblk[:],
                        xT[:, s4 * NQ : (s4 + 1) * NQ].rearrange(
                            "(dc p) s -> p dc s", p=P
                        ),
                    )
                    for j in range(JCH):
                        ps = pp.tile([P, NQ], F32, tag="pp")
                        for d in range(DC):
                            mm(
                                ps[:],
                                w_sb[:, d, j * P : (j + 1) * P],
                                xblk[:, d, :],
                                start=(d == 0),
                                stop=(d == DC - 1),
                            )
                        nc.scalar.activation(
                            dst[:, j, s4 * NQ : (s4 + 1) * NQ],
                            ps[:],
                            AF.Identity,
                            bias=b_sb[:, j : j + 1],
                        )

            # v: out chunks [s 128, j 512] = xvT.T @ wvT (+bv via K=1 matmul)
            w_sb = wpool.tile([P, DC, JCA], DT, tag="w")
            nc.sync.dma_start(w_sb[:], wvT.rearrange("(dc p) j -> p dc j", p=P))
            for s4 in range(NQC):
                xblk = xpool.tile([P, DC, NQ], DT, tag="x")
                nc.sync.dma_start(
                    xblk[:],
                    xvT[:, s4 * NQ : (s4 + 1) * NQ].rearrange("(dc p) s -> p dc s", p=P),
                )
                for s16 in range(4):
                    sc = s4 * 4 + s16
                    ps = ppv.tile([P, 2, NQ], F32, tag="ppv")
                    HJ = JCA // 2  # 260
                    for half in range(2):
                        for d in range(DC):
                            mm(
                                ps[:, half, 0:HJ],
                                xblk[:, d, s16 * P : (s16 + 1) * P],
                                w_sb[:, d, half * HJ : (half + 1) * HJ],
                                start=(d == 0),
                                stop=False,
                            )
                        mm(
                            ps[:, half, 0:HJ],
                            ones_c[:],
                            bv_row[:, half * HJ : (half + 1) * HJ],
                            start=False,
                            stop=True,
                        )
                    nc.scalar.activation(
                        v_sb[:, sc].rearrange("p (a b) e -> p a (b e)", a=2),
                        ps[:, :, 0:HJ],
                        AF.Copy,
                    )

        # ---- stages B+C: attention + output projection ----
        with ExitStack() as attn:
            mpool = attn.enter_context(tc.tile_pool(name="mp", bufs=1))
            epool = attn.enter_context(tc.tile_pool(name="ep", bufs=3))
            spool = attn.enter_context(tc.tile_pool(name="sp", bufs=3, space="PSUM"))
            opool = attn.enter_context(tc.tile_pool(name="op", bufs=2, space="PSUM"))
            rpool = attn.enter_context(tc.tile_pool(name="rp", bufs=2))
            obuf = attn.enter_context(tc.tile_pool(name="ob", bufs=2))
            if n_reps > 1:
                rep_b = attn.enter_context(tc.For_i(0, n_reps, 1))

            for qc in range(NQC):
                mblk = mpool.tile([P, SC, NQ], MASKDT, tag="m")
                nc.sync.dma_start(
                    mblk[:],
                    maskT[:, qc * NQ : (qc + 1) * NQ].rearrange(
                        "(kc p) q -> p kc q", p=P
                    ),
                )
                for hp in range(4):
                    o_ps0 = opool.tile([HD + 1, NQ], F32, tag="ot")
                    o_ps1 = opool.tile([HD + 1, NQ], F32, tag="ot")
                    o_ps = (o_ps0, o_ps1)
                    for kg in range(NKG):
                        e_ts = []
                        for hi in range(2):
                            h = 2 * hp + hi
                            r0 = (h % 2) * HD
                            ch = h // 2
                            sc_ps = spool.tile([P, KG, NQ], F32, tag="sc")
                            for k2 in range(KG):
                                kc = kg * KG + k2
                                mm(
                                    sc_ps[:, k2, :],
                                    kT_sb[r0 : r0 + HD, ch, kc * P : (kc + 1) * P],
                                    qT_sb[r0 : r0 + HD, ch, qc * NQ : (qc + 1) * NQ],
                                    start=True,
                                    stop=True,
                                )
                            me = epool.tile([P, KG, NQ], F32, tag="me")
                            nc.vector.tensor_tensor(
                                me[:],
                                sc_ps[:],
                                mblk[:, kg * KG : (kg + 1) * KG, :],
                                MUL,
                            )
                            et = epool.tile([P, KG, NQ], DT, tag="et")
                            nc.scalar.activation(et[:], me[:], AF.Exp)
                            e_ts.append(et)
                        for hi in range(2):
                            h = 2 * hp + hi
                            for k2 in range(KG):
                                kc = kg * KG + k2
                                mm(
                                    o_ps[hi][:],
                                    v_sb[:, kc, h, :],
                                    e_ts[hi][:, k2, :],
                                    start=(kg == 0 and k2 == 0),
                                    stop=(kg == NKG - 1 and k2 == KG - 1),
                                )
                    for hi in range(2):
                        h = 2 * hp + hi
                        r0 = (h % 2) * HD
                        ch = h // 2
                        recip = rpool.tile([1, NQ], F32, tag="rc")
                        nc.vector.reciprocal(recip[:], o_ps[hi][HD : HD + 1, :])
                        rb_ps = spool.tile([HD, NQ], F32, tag="sc")
                        mm(rb_ps[:], ones_c[:, 0:HD], recip[:], start=True, stop=True)
                        rb_sb = rpool.tile([HD, NQ], F32, tag="rb")
                        nc.scalar.activation(rb_sb[:], rb_ps[:], AF.Copy)
                        nc.vector.tensor_tensor(
                            outnT_sb[r0 : r0 + HD, ch, qc * NQ : (qc + 1) * NQ],
                            o_ps[hi][0:HD, :],
                            rb_sb[:],
                            MUL,
                        )

            # ---- stage C: output projection ----
            for sc in range(SC):
                fps = spool.tile([P, 2, NQ], F32, tag="sc")
                for n2 in range(2):
                    for cc in range(JCH):
                        mm(
                            fps[:, n2, :],
                            outnT_sb[:, cc, sc * P : (sc + 1) * P],
                            woT_sb[:, cc, n2 * NQ : (n2 + 1) * NQ],
                            start=(cc == 0),
                            stop=(cc == JCH - 1),
                        )
                ob = obuf.tile([P, D], F32, tag="o")
                nc.scalar.activation(ob[:], fps[:].rearrange("p a b -> p (a b)"), AF.Copy)
                nc.sync.dma_start(outp[sc * P : (sc + 1) * P, :], ob[:])

    nc.compile()
    return nc


def _augment_wv(WvJ, f):
    # [512, 1024] row-slice -> transposed + per-head ones column -> [1024, 520]
    out = np.zeros((D, JCA), dtype=f)
    wt = WvJ.T  # [1024, 512]
    for h in range(8):
        out[:, h * (HD + 1) : h * (HD + 1) + HD] = wt[:, h * HD : (h + 1) * HD]
    return out


def _augment_bv(bvJ, f):
    out = np.zeros(JCA, dtype=f)
    for h in range(8):
        out[h * (HD + 1) : h * (HD + 1) + HD] = bvJ[h * HD : (h + 1) * HD]
        out[h * (HD + 1) + HD] = 1.0
    return out


def _prep_in_maps(query, key, value, mask, Wq, bq, Wk, bk, Wv, bv, Wo, bo):
    f = np.float32
    per_batch = []
    for b in range(B):
        per_batch.append(
            dict(
                xqT=np.ascontiguousarray(query[b].T, dtype=f),
                xkT=np.ascontiguousarray(key[b].T, dtype=f),
                xvT=np.ascontiguousarray(value[b].T, dtype=f),
                maskT=np.ascontiguousarray(mask[b, 0].T).astype(_NP_MASKDT),
            )
        )
    per_half = []
    for hh in range(2):
        J = slice(JC * hh, JC * (hh + 1))
        per_half.append(
            dict(
                wqT=np.ascontiguousarray(Wq[J].T, dtype=f),
                wkT=np.ascontiguousarray(Wk[J].T, dtype=f),
                wvT=_augment_wv(Wv[J], f),
                bq=np.ascontiguousarray(bq[J], dtype=f),
                bk=np.ascontiguousarray(bk[J], dtype=f),
                bv=_augment_bv(bv[J], f),
                woT=np.ascontiguousarray(Wo[:, J].T, dtype=f),
            )
        )
    in_maps = []
    for c in range(8):
        m = dict(per_batch[c // 2])
        m.update(per_half[c % 2])
        in_maps.append(m)
    return in_maps


_NC_CACHE = {}


def _get_nc(dt_name="float32r", n_reps=1):
    key = (dt_name, n_reps)
    if key not in _NC_CACHE:
        _NC_CACHE[key] = build_nc(DT=getattr(mybir.dt, dt_name), n_reps=n_reps)
    return _NC_CACHE[key]


# ---------------------------------------------------------------------------
# Cached PJRT runner.  Mirrors run_bass_kernel_spmd's axon redirect
# (bass2jax.run_bass_via_pjrt) but builds the jitted shard_map once per
# (dt_name, n_execs) so repeat kernel() calls skip re-tracing, and supports
# chaining n_execs sequential NEFF executions inside one program so test.py
# can measure per-execution hardware time as a slope (dispatch overhead
# cancels).
# ---------------------------------------------------------------------------
_RUNNER_CACHE = {}


def _get_runner(dt_name="float32r", n_reps=1):
    key = (dt_name, n_reps)
    if key in _RUNNER_CACHE:
        return _RUNNER_CACHE[key]

    import jax
    from jax.sharding import Mesh, PartitionSpec
    from jax.experimental.shard_map import shard_map
    from concourse import bass2jax
    from concourse.bass2jax import _bass_exec_p

    bass2jax.install_neuronx_cc_hook()
    nc = _get_nc(dt_name, n_reps)
    partition_name = nc.partition_id_tensor.name if nc.partition_id_tensor else None

    in_names = []
    out_names = []
    out_avals = []
    for alloc in nc.m.functions[0].allocations:
        if not isinstance(alloc, mybir.MemoryLocationSet):
            continue
        name = alloc.memorylocations[0].name
        if alloc.kind == "ExternalInput":
            if name != partition_name:
                in_names.append(name)
        elif alloc.kind == "ExternalOutput":
            out_names.append(name)
            out_avals.append(
                jax.core.ShapedArray(tuple(alloc.tensor_shape), mybir.dt.np(alloc.dtype))
            )
    n_params = len(in_names)
    n_outs = len(out_avals)
    all_in_names = tuple(in_names + out_names)
    if partition_name is not None:
        all_in_names = all_in_names + (partition_name,)

    def _body(*args):
        params = list(args[:n_params])
        zeros = list(args[n_params:])
        pid = (
            [bass2jax.partition_id_tensor()] if partition_name is not None else []
        )
        outs = _bass_exec_p.bind(
            *params,
            *zeros,
            *pid,
            out_avals=tuple(out_avals),
            in_names=all_in_names,
            out_names=tuple(out_names),
            lowering_input_output_aliases=(),
            sim_require_finite=True,
            sim_require_nnan=True,
            nc=nc,
        )
        return tuple(outs)

    devices = jax.devices()[:8]
    mesh = Mesh(np.asarray(devices), ("core",))
    in_specs = (PartitionSpec("core"),) * (n_params + n_outs)
    out_specs = (PartitionSpec("core"),) * n_outs
    fn = jax.jit(
        shard_map(_body, mesh=mesh, in_specs=in_specs, out_specs=out_specs,
                  check_rep=False),
        keep_unused=True,
    )
    runner = (fn, in_names, out_names, out_avals)
    _RUNNER_CACHE[key] = runner
    return runner


def _concat_inputs(in_maps, in_names, out_avals, out_names):
    args = []
    for name in in_names:
        args.append(np.concatenate([np.asarray(m[name]) for m in in_maps], axis=0))
    for i, name in enumerate(out_names):
        z = out_avals[i]
        args.append(np.zeros((8 * z.shape[0], *z.shape[1:]), z.dtype))
    return args


def run(inputs, dt_name="float32r"):
    """Returns (full_output [B,S,D] f32, per-core outp list)."""
    fn, in_names, out_names, out_avals = _get_runner(dt_name, 1)
    in_maps = _prep_in_maps(**inputs)
    args = _concat_inputs(in_maps, in_names, out_avals, out_names)
    out_arrs = fn(*args)
    i = out_names.index("outp")
    per_core = np.asarray(out_arrs[i]).reshape(8, S, D)
    bo = np.asarray(inputs["bo"], dtype=np.float32)
    out = np.empty((B, S, D), dtype=np.float32)
    for b in range(B):
        out[b] = per_core[2 * b] + per_core[2 * b + 1] + bo
    return out, per_core


def bench(inputs, dt_name="float32r", n_reps=1, iters=6):
    """Time the NEFF whose body repeats n_reps times on-device."""
    import time as _time
    import jax
    fn, in_names, out_names, out_avals = _get_runner(dt_name, n_reps)
    in_maps = _prep_in_maps(**inputs)
    args = _concat_inputs(in_maps, in_names, out_avals, out_names)
    dargs = [jax.device_put(a) for a in args]
    times = []
    for _ in range(iters):
        t0 = _time.perf_counter()
        outs = fn(*dargs)
        jax.block_until_ready(outs)
        times.append(_time.perf_counter() - t0)
    return times


_TRIVIAL = {}


def _get_trivial():
    """A minimal 8-core NEFF (one small copy) to measure dispatch overhead."""
    if "nc" in _TRIVIAL:
        return _TRIVIAL["nc"]
    nc = bacc.Bacc("TRN2", target_bir_lowering=False, debug=False)
    a = nc.dram_tensor("a", [P, NQ], F32, kind="ExternalInput").ap()
    o = nc.dram_tensor("o", [P, NQ], F32, kind="ExternalOutput").ap()
    with tile.TileContext(nc) as tc, ExitStack() as ctx:
        sb = ctx.enter_context(tc.tile_pool(name="sb", bufs=1))
        t = sb.tile([P, NQ], F32)
        nc.sync.dma_start(t[:], a[:])
        nc.sync.dma_start(o[:], t[:])
    nc.compile()
    _TRIVIAL["nc"] = nc
    return nc


def bench_trivial(iters=8):
    import time as _time
    import jax
    from jax.sharding import Mesh, PartitionSpec
    from jax.experimental.shard_map import shard_map
    from concourse import bass2jax
    from concourse.bass2jax import _bass_exec_p

    if "fn" not in _TRIVIAL:
        bass2jax.install_neuronx_cc_hook()
        nc = _get_trivial()
        partition_name = nc.partition_id_tensor.name if nc.partition_id_tensor else None
        out_avals = (jax.core.ShapedArray((P, NQ), np.float32),)
        all_in = ("a", "o") + ((partition_name,) if partition_name else ())

        def _body(a, z):
            pid = [bass2jax.partition_id_tensor()] if partition_name else []
            return tuple(_bass_exec_p.bind(
                a, z, *pid,
                out_avals=out_avals,
                in_names=all_in,
                out_names=("o",),
                lowering_input_output_aliases=(),
                sim_require_finite=True,
                sim_require_nnan=True,
                nc=nc,
            ))

        devices = jax.devices()[:8]
        mesh = Mesh(np.asarray(devices), ("core",))
        fn = jax.jit(
            shard_map(
                _body, mesh=mesh,
                in_specs=(PartitionSpec("core"),) * 2,
                out_specs=(PartitionSpec("core"),),
                check_rep=False,
            ),
            keep_unused=True,
        )
        _TRIVIAL["fn"] = fn
    fn = _TRIVIAL["fn"]
    import jax
    A = jax.device_put(np.zeros((8 * P, NQ), np.float32))
    Z = jax.device_put(np.zeros((8 * P, NQ), np.float32))
    times = []
    for _ in range(iters):
        t0 = _time.perf_counter()
        outs = fn(A, Z)
        jax.block_until_ready(outs)
        times.append(_time.perf_counter() - t0)
    return times


def kernel(**inputs):
    out, _ = run(inputs)
    return out


# revision 23
# speedup vs baseline: 15.4178x; 1.5004x over previous
"""Multi-head attention on 8 Trainium2 NeuronCores (Bass/Tile).

Sharding: core c handles batch b = c//2 and head-half hh = c%2
(heads 8*hh..8*hh+7, i.e. 512 of the 1024 hidden columns).
Each core computes its half-head attention plus the partial output
projection through Wo[:, cols_half]; the host sums the two partials
per batch and adds bo.

Per-core device program: see build_nc() docstring.
"""

import numpy as np
from contextlib import ExitStack

import ml_dtypes

import concourse.bacc as bacc
import concourse.mybir as mybir
import concourse.tile as tile
from concourse.bass_utils import run_bass_kernel_spmd

F32 = mybir.dt.float32
AF = mybir.ActivationFunctionType
MUL = mybir.AluOpType.mult

B, S, D, H = 4, 2048, 1024, 16
HD = 64
P = 128
JC = 512          # head-cols per core
DC = D // P       # 8 d-chunks
JCH = JC // P     # 4 j-chunks
SC = S // P       # 16 s/k chunks of 128
NQ = 512          # q processed in chunks of 512
NQC = S // NQ     # 4
KG = 2            # k-chunks per group (DVE/ACT op width 1024)
NKG = SC // KG    # 8
JCA = 8 * (HD + 1)  # 520: v-proj output cols, ones column per head baked in

MASKDT = mybir.dt.bfloat16
_NP_MASKDT = ml_dtypes.bfloat16


def build_nc(DT=mybir.dt.float32r, maskdt=MASKDT, n_reps=1, rep_stage="both", cfg=None):
    """One NeuronCore's program: projections -> attention -> output projection."""
    cfg = cfg or {}
    nc = bacc.Bacc("TRN2", target_bir_lowering=False, debug=False)

    def mm(out, lhsT, rhs, **kw):
        nc.tensor.matmul(out, lhsT, rhs, **kw)

    xqT = nc.dram_tensor("xqT", [D, S], DT, kind="ExternalInput").ap()
    xkT = nc.dram_tensor("xkT", [D, S], DT, kind="ExternalInput").ap()
    xvT = nc.dram_tensor("xvT", [D, S], DT, kind="ExternalInput").ap()
    wqT = nc.dram_tensor("wqT", [D, JC], DT, kind="ExternalInput").ap()
    wkT = nc.dram_tensor("wkT", [D, JC], DT, kind="ExternalInput").ap()
    wvT = nc.dram_tensor("wvT", [D, JCA], DT, kind="ExternalInput").ap()
    bq = nc.dram_tensor("bq", [JC], F32, kind="ExternalInput").ap()
    bk = nc.dram_tensor("bk", [JC], F32, kind="ExternalInput").ap()
    bv = nc.dram_tensor("bv", [JCA], F32, kind="ExternalInput").ap()
    woT = nc.dram_tensor("woT", [JC, D], DT, kind="ExternalInput").ap()
    maskT = nc.dram_tensor("maskT", [S, S], maskdt, kind="ExternalInput").ap()
    outp = nc.dram_tensor("outp", [S, D], F32, kind="ExternalOutput").ap()

    with (
        tile.TileContext(nc) as tc,
        nc.allow_low_precision(reason="float32r operand storage for PE fast path"),
        ExitStack() as persist,
    ):
        # ---- long-lived SBUF ----
        pers = persist.enter_context(tc.tile_pool(name="pers", bufs=1))
        qT_sb = pers.tile([P, JCH, S], DT)          # [j%128, jc, s]
        kT_sb = pers.tile([P, JCH, S], DT)
        v_sb = pers.tile([P, SC, 8, HD + 1], DT)    # [s%128, sc, head, hd+ones]
        brow = pers.tile([1, P + JCA], F32)         # [ones_c | bv_row]
        bqk = pers.tile([P, 2 * JCH], F32)          # [bq | bk] per-partition bias
        ones_c = brow[:, 0:P]
        bv_row = brow[:, P : P + JCA]
        bq_sb = bqk[:, 0:JCH]
        bk_sb = bqk[:, JCH : 2 * JCH]

        nc.vector.memset(ones_c, 1.0)
        nc.sync.dma_start(bv_row, bv.rearrange("(o j) -> o j", o=1))
        nc.sync.dma_start(bq_sb, bq.rearrange("(c p) -> p c", p=P))
        nc.sync.dma_start(bk_sb, bk.rearrange("(c p) -> p c", p=P))

        # ---- stage A: projections ----
        with ExitStack() as proj:
            xpool = proj.enter_context(tc.tile_pool(name="xp", bufs=cfg.get("xbufs", 3)))
            wpool = proj.enter_context(tc.tile_pool(name="wp", bufs=cfg.get("wbufs", 2)))
            pp = proj.enter_context(tc.tile_pool(name="pp", bufs=4, space="PSUM"))
            ppv = proj.enter_context(tc.tile_pool(name="ppv", bufs=2, space="PSUM"))
            if n_reps > 1 and rep_stage in ("both", "proj"):
                rep_a = proj.enter_context(tc.For_i(0, n_reps, 1))

            # q and k: out chunks [j 128, s 512] = wT.T @ xT  (+bias on ACT copy)
            for name, xT, wT, b_sb, dst in (
                ("q", xqT, wqT, bq_sb, qT_sb),
                ("k", xkT, wkT, bk_sb, kT_sb),
            ):
                w_sb = wpool.tile([P, DC, JCA], DT, tag="w")
                nc.sync.dma_start(
                    w_sb[:, :, 0:JC], wT.rearrange("(dc p) j -> p dc j", p=P)
                )
                for s4 in range(NQC):
                    xblk = xpool.tile([P, DC, NQ], DT, tag="x")
                    nc.sync.dma_start(
                        xblk[:],
                        xT[:, s4 * NQ : (s4 + 1) * NQ].rearrange(
                            "(dc p) s -> p dc s", p=P
                        ),
                    )
                    for j in range(JCH):
                        ps = pp.tile([P, NQ], F32, tag="pp")
                        for d in range(DC):
                            mm(
                                ps[:],
                                w_sb[:, d, j * P : (j + 1) * P],
                                xblk[:, d, :],
                                start=(d == 0),
                                stop=(d == DC - 1),
                            )
                        nc.scalar.activation(
                            dst[:, j, s4 * NQ : (s4 + 1) * NQ],
                            ps[:],
                            AF.Identity,
                            bias=b_sb[:, j : j + 1],
                        )

            # v: out chunks [s 128, j 520] = xvT.T @ wvT_aug (+bv/ones via K=1)
            w_sb = wpool.tile([P, DC, JCA], DT, tag="w")
            nc.sync.dma_start(w_sb[:], wvT.rearrange("(dc p) j -> p dc j", p=P))
            HJ = JCA // 2  # 260
            for s4 in range(NQC):
                xblk = xpool.tile([P, DC, NQ], DT, tag="x")
                nc.sync.dma_start(
                    xblk[:],
                    xvT[:, s4 * NQ : (s4 + 1) * NQ].rearrange("(dc p) s -> p dc s", p=P),
                )
                for s16 in range(4):
                    sc = s4 * 4 + s16
                    ps = ppv.tile([P, 2, NQ], F32, tag="ppv")
                    for half in range(2):
                        for d in range(DC):
                            mm(
                                ps[:, half, 0:HJ],
                                xblk[:, d, s16 * P : (s16 + 1) * P],
                                w_sb[:, d, half * HJ : (half + 1) * HJ],
                                start=(d == 0),
                                stop=False,
                            )
                        mm(
                            ps[:, half, 0:HJ],
                            ones_c,
                            bv_row[:, half * HJ : (half + 1) * HJ],
                            start=False,
                            stop=True,
                        )
                    nc.scalar.activation(
                        v_sb[:, sc].rearrange("p (a b) e -> p a (b e)", a=2),
                        ps[:, :, 0:HJ],
                        AF.Copy,
                    )

        # ---- stages B+C: attention + output projection ----
        with ExitStack() as attn:
            wopool = attn.enter_context(tc.tile_pool(name="wop", bufs=1))
            outnT_sb = wopool.tile([P, JCH, S], DT)     # normalized head_out.T
            woT_sb = wopool.tile([P, JCH, D], DT)
            nc.sync.dma_start(woT_sb[:], woT.rearrange("(c p) j -> p c j", p=P))
            mpool = attn.enter_context(tc.tile_pool(name="mp", bufs=cfg.get("mbufs", 2)))
            epool = attn.enter_context(tc.tile_pool(name="ep", bufs=cfg.get("ebufs", 3)))
            spool = attn.enter_context(
                tc.tile_pool(name="sp", bufs=cfg.get("sbufs", 3), space="PSUM")
            )
            opool = attn.enter_context(tc.tile_pool(name="op", bufs=cfg.get("obufs", 2), space="PSUM"))
            rpool = attn.enter_context(tc.tile_pool(name="rp", bufs=cfg.get("rbufs", 2)))
            obuf = attn.enter_context(tc.tile_pool(name="ob", bufs=2))
            if n_reps > 1 and rep_stage in ("both", "attn"):
                rep_b = attn.enter_context(tc.For_i(0, n_reps, 1))

            for qc in range(NQC):
                mblk = mpool.tile([P, SC, NQ], MASKDT, tag="m")
                nc.sync.dma_start(
                    mblk[:],
                    maskT[:, qc * NQ : (qc + 1) * NQ].rearrange(
                        "(kc p) q -> p kc q", p=P
                    ),
                )
                for hp in range(4):
                    o_ps0 = opool.tile([HD + 1, NQ], F32, tag="ot")
                    o_ps1 = opool.tile([HD + 1, NQ], F32, tag="ot")
                    o_ps = (o_ps0, o_ps1)
                    for kg in range(NKG):
                        e_ts = []
                        for hi in range(2):
                            h = 2 * hp + hi
                            r0 = (h % 2) * HD
                            ch = h // 2
                            sc_ps = spool.tile([P, KG, NQ], F32, tag="sc")
                            for k2 in range(KG):
                                kc = kg * KG + k2
                                mm(
                                    sc_ps[:, k2, :],
                                    kT_sb[r0 : r0 + HD, ch, kc * P : (kc + 1) * P],
                                    qT_sb[r0 : r0 + HD, ch, qc * NQ : (qc + 1) * NQ],
                                    start=True,
                                    stop=True,
                                )
                            # mask multiply (DVE) then exp in place (ACT)
                            me = epool.tile([P, KG, NQ], DT, tag="me")
                            nc.vector.tensor_tensor(
                                me[:],
                                sc_ps[:],
                                mblk[:, kg * KG : (kg + 1) * KG, :],
                                MUL,
                            )
                            nc.scalar.activation(me[:], me[:], AF.Exp)
                            e_ts.append(me)
                        for hi in range(2):
                            h = 2 * hp + hi
                            for k2 in range(KG):
                                kc = kg * KG + k2
                                mm(
                                    o_ps[hi][:],
                                    v_sb[:, kc, h, :],
                                    e_ts[hi][:, k2, :],
                                    start=(kg == 0 and k2 == 0),
                                    stop=(kg == NKG - 1 and k2 == KG - 1),
                                )
                    for hi in range(2):
                        h = 2 * hp + hi
                        r0 = (h % 2) * HD
                        ch = h // 2
                        recip = rpool.tile([1, NQ], F32, tag="rc")
                        nc.vector.reciprocal(recip[:], o_ps[hi][HD : HD + 1, :])
                        rb_ps = spool.tile([HD, NQ], F32, tag="sc")
                        mm(rb_ps[:], ones_c[:, 0:HD], recip[:], start=True, stop=True)
                        rb_sb = rpool.tile([HD, NQ], F32, tag="rb")
                        nc.scalar.activation(rb_sb[:], rb_ps[:], AF.Copy)
                        nc.vector.tensor_tensor(
                            outnT_sb[r0 : r0 + HD, ch, qc * NQ : (qc + 1) * NQ],
                            o_ps[hi][0:HD, :],
                            rb_sb[:],
                            MUL,
                        )

            # ---- stage C: output projection ----
            for sc in range(SC):
                fps = spool.tile([P, 2, NQ], F32, tag="sc")
                for n2 in range(2):
                    for cc in range(JCH):
                        mm(
                            fps[:, n2, :],
                            outnT_sb[:, cc, sc * P : (sc + 1) * P],
                            woT_sb[:, cc, n2 * NQ : (n2 + 1) * NQ],
                            start=(cc == 0),
                            stop=(cc == JCH - 1),
                        )
                ob = obuf.tile([P, D], F32, tag="o")
                nc.scalar.activation(ob[:], fps[:].rearrange("p a b -> p (a b)"), AF.Copy)
                nc.sync.dma_start(outp[sc * P : (sc + 1) * P, :], ob[:])

    nc.compile()
    return nc

def _augment_wv(WvJ, f):
    # [512, 1024] row-slice -> transposed + per-head ones column -> [1024, 520]
    out = np.zeros((D, JCA), dtype=f)
    wt = WvJ.T  # [1024, 512]
    for h in range(8):
        out[:, h * (HD + 1) : h * (HD + 1) + HD] = wt[:, h * HD : (h + 1) * HD]
    return out


def _augment_bv(bvJ, f):
    out = np.zeros(JCA, dtype=f)
    for h in range(8):
        out[h * (HD + 1) : h * (HD + 1) + HD] = bvJ[h * HD : (h + 1) * HD]
        out[h * (HD + 1) + HD] = 1.0
    return out


def _prep_in_maps(query, key, value, mask, Wq, bq, Wk, bk, Wv, bv, Wo, bo):
    f = np.float32
    per_batch = []
    for b in range(B):
        per_batch.append(
            dict(
                xqT=np.ascontiguousarray(query[b].T, dtype=f),
                xkT=np.ascontiguousarray(key[b].T, dtype=f),
                xvT=np.ascontiguousarray(value[b].T, dtype=f),
                maskT=np.ascontiguousarray(mask[b, 0].T).astype(_NP_MASKDT),
            )
        )
    per_half = []
    for hh in range(2):
        J = slice(JC * hh, JC * (hh + 1))
        per_half.append(
            dict(
                wqT=np.ascontiguousarray(Wq[J].T, dtype=f),
                wkT=np.ascontiguousarray(Wk[J].T, dtype=f),
                wvT=_augment_wv(Wv[J], f),
                bq=np.ascontiguousarray(bq[J], dtype=f),
                bk=np.ascontiguousarray(bk[J], dtype=f),
                bv=_augment_bv(bv[J], f),
                woT=np.ascontiguousarray(Wo[:, J].T, dtype=f),
            )
        )
    in_maps = []
    for c in range(8):
        m = dict(per_batch[c // 2])
        m.update(per_half[c % 2])
        in_maps.append(m)
    return in_maps


_NC_CACHE = {}


def _get_nc(dt_name="float32r", n_reps=1, rep_stage="both"):
    key = (dt_name, n_reps, rep_stage)
    if key not in _NC_CACHE:
        _NC_CACHE[key] = build_nc(
            DT=getattr(mybir.dt, dt_name), n_reps=n_reps, rep_stage=rep_stage
        )
    return _NC_CACHE[key]


# ---------------------------------------------------------------------------
# Cached PJRT runner.  Mirrors run_bass_kernel_spmd's axon redirect
# (bass2jax.run_bass_via_pjrt) but builds the jitted shard_map once per
# (dt_name, n_execs) so repeat kernel() calls skip re-tracing, and supports
# chaining n_execs sequential NEFF executions inside one program so test.py
# can measure per-execution hardware time as a slope (dispatch overhead
# cancels).
# ---------------------------------------------------------------------------
_RUNNER_CACHE = {}


def _get_runner(dt_name="float32r", n_reps=1, rep_stage="both"):
    key = (dt_name, n_reps, rep_stage)
    if key in _RUNNER_CACHE:
        return _RUNNER_CACHE[key]

    import jax
    from jax.sharding import Mesh, PartitionSpec
    from jax.experimental.shard_map import shard_map
    from concourse import bass2jax
    from concourse.bass2jax import _bass_exec_p

    bass2jax.install_neuronx_cc_hook()
    nc = _get_nc(dt_name, n_reps, rep_stage)
    partition_name = nc.partition_id_tensor.name if nc.partition_id_tensor else None

    in_names = []
    out_names = []
    out_avals = []
    for alloc in nc.m.functions[0].allocations:
        if not isinstance(alloc, mybir.MemoryLocationSet):
            continue
        name = alloc.memorylocations[0].name
        if alloc.kind == "ExternalInput":
            if name != partition_name:
                in_names.append(name)
        elif alloc.kind == "ExternalOutput":
            out_names.append(name)
            out_avals.append(
                jax.core.ShapedArray(tuple(alloc.tensor_shape), mybir.dt.np(alloc.dtype))
            )
    n_params = len(in_names)
    n_outs = len(out_avals)
    all_in_names = tuple(in_names + out_names)
    if partition_name is not None:
        all_in_names = all_in_names + (partition_name,)

    def _body(*args):
        params = list(args[:n_params])
        zeros = list(args[n_params:])
        pid = (
            [bass2jax.partition_id_tensor()] if partition_name is not None else []
        )
        outs = _bass_exec_p.bind(
            *params,
            *zeros,
            *pid,
            out_avals=tuple(out_avals),
            in_names=all_in_names,
            out_names=tuple(out_names),
            lowering_input_output_aliases=(),
            sim_require_finite=True,
            sim_require_nnan=True,
            nc=nc,
        )
        return tuple(outs)

    devices = jax.devices()[:8]
    mesh = Mesh(np.asarray(devices), ("core",))
    in_specs = (PartitionSpec("core"),) * (n_params + n_outs)
    out_specs = (PartitionSpec("core"),) * n_outs
    fn = jax.jit(
        shard_map(_body, mesh=mesh, in_specs=in_specs, out_specs=out_specs,
                  check_rep=False),
        keep_unused=True,
    )
    runner = (fn, in_names, out_names, out_avals)
    _RUNNER_CACHE[key] = runner
    return runner


def _concat_inputs(in_maps, in_names, out_avals, out_names):
    args = []
    for name in in_names:
        args.append(np.concatenate([np.asarray(m[name]) for m in in_maps], axis=0))
    for i, name in enumerate(out_names):
        z = out_avals[i]
        args.append(np.zeros((8 * z.shape[0], *z.shape[1:]), z.dtype))
    return args


def run(inputs, dt_name="float32r"):
    """Returns (full_output [B,S,D] f32, per-core outp list)."""
    fn, in_names, out_names, out_avals = _get_runner(dt_name, 1)
    in_maps = _prep_in_maps(**inputs)
    args = _concat_inputs(in_maps, in_names, out_avals, out_names)
    out_arrs = fn(*args)
    i = out_names.index("outp")
    per_core = np.asarray(out_arrs[i]).reshape(8, S, D)
    bo = np.asarray(inputs["bo"], dtype=np.float32)
    out = np.empty((B, S, D), dtype=np.float32)
    for b in range(B):
        out[b] = per_core[2 * b] + per_core[2 * b + 1] + bo
    return out, per_core


def bench(inputs, dt_name="float32r", n_reps=1, iters=6, rep_stage="both"):
    """Time the NEFF whose body repeats n_reps times on-device."""
    import time as _time
    import jax
    fn, in_names, out_names, out_avals = _get_runner(dt_name, n_reps, rep_stage)
    in_maps = _prep_in_maps(**inputs)
    args = _concat_inputs(in_maps, in_names, out_avals, out_names)
    dargs = [jax.device_put(a) for a in args]
    times = []
    for _ in range(iters):
        t0 = _time.perf_counter()
        outs = fn(*dargs)
        jax.block_until_ready(outs)
        times.append(_time.perf_counter() - t0)
    return times


_TRIVIAL = {}


def _get_trivial():
    """A minimal 8-core NEFF (one small copy) to measure dispatch overhead."""
    if "nc" in _TRIVIAL:
        return _TRIVIAL["nc"]
    nc = bacc.Bacc("TRN2", target_bir_lowering=False, debug=False)
    a = nc.dram_tensor("a", [P, NQ], F32, kind="ExternalInput").ap()
    o = nc.dram_tensor("o", [P, NQ], F32, kind="ExternalOutput").ap()
    with tile.TileContext(nc) as tc, ExitStack() as ctx:
        sb = ctx.enter_context(tc.tile_pool(name="sb", bufs=1))
        t = sb.tile([P, NQ], F32)
        nc.sync.dma_start(t[:], a[:])
        nc.sync.dma_start(o[:], t[:])
    nc.compile()
    _TRIVIAL["nc"] = nc
    return nc


def bench_trivial(iters=8):
    import time as _time
    import jax
    from jax.sharding import Mesh, PartitionSpec
    from jax.experimental.shard_map import shard_map
    from concourse import bass2jax
    from concourse.bass2jax import _bass_exec_p

    if "fn" not in _TRIVIAL:
        bass2jax.install_neuronx_cc_hook()
        nc = _get_trivial()
        partition_name = nc.partition_id_tensor.name if nc.partition_id_tensor else None
        out_avals = (jax.core.ShapedArray((P, NQ), np.float32),)
        all_in = ("a", "o") + ((partition_name,) if partition_name else ())

        def _body(a, z):
            pid = [bass2jax.partition_id_tensor()] if partition_name else []
            return tuple(_bass_exec_p.bind(
                a, z, *pid,
                out_avals=out_avals,
                in_names=all_in,
                out_names=("o",),
                lowering_input_output_aliases=(),
                sim_require_finite=True,
                sim_require_nnan=True,
                nc=nc,
            ))

        devices = jax.devices()[:8]
        mesh = Mesh(np.asarray(devices), ("core",))
        fn = jax.jit(
            shard_map(
                _body, mesh=mesh,
                in_specs=(PartitionSpec("core"),) * 2,
                out_specs=(PartitionSpec("core"),),
                check_rep=False,
            ),
            keep_unused=True,
        )
        _TRIVIAL["fn"] = fn
    fn = _TRIVIAL["fn"]
    import jax
    A = jax.device_put(np.zeros((8 * P, NQ), np.float32))
    Z = jax.device_put(np.zeros((8 * P, NQ), np.float32))
    times = []
    for _ in range(iters):
        t0 = _time.perf_counter()
        outs = fn(A, Z)
        jax.block_until_ready(outs)
        times.append(_time.perf_counter() - t0)
    return times


def kernel(**inputs):
    inputs = {k: np.asarray(v) for k, v in inputs.items()}
    out, _ = run(inputs)
    return out
